# revision 25
# baseline (speedup 1.0000x reference)
"""Trainium2 Bass kernel for a 4-layer GCN (nn_GCNModel_44762149159246).

Reference math (per batch b of B=4096, N=128 nodes, F=67 in-feats, H=64):
    h0 = node @ W_emb                                  [N, H]
    for l in 0..3:  h = relu(sigmoid(adj @ (h W_l + b_l)) + h)
    out = sigmoid(sum_n h @ W_fc + b_fc)               scalar per batch

Sharding: pure data-parallel over the batch dim, 512 batches per core x 8.

Device-side layout (everything transpose-free on device):
  - state is kept TRANSPOSED and PAIR-PACKED: tile [128, n] where partitions
    0..63 hold features of the even batch of a pair and 64..127 the odd batch.
  - adj is passed host-transposed per batch ([m, b, n] so that adj.T tiles
    stream naturally as the matmul moving operand).
  - node is passed pair-packed/transposed ([feat-pair, pair, n]) with the
    3 leftover features (67 = 64 + 3) in a small side tensor, contracted via
    a second accumulating matmul.
  - per-layer W is a 128x128 block-diagonal [[W, 0], [0, W]] so one matmul
    computes both halves of a pair; the bias is added during the PSUM->SBUF
    copy with a broadcast tile.
  - relu is provably a no-op for layers 1..3 (sigmoid > 0 and h >= 0), so it
    is only applied after layer 0.
"""

import os
import sys

import numpy as np

for _p in ("/opt/trn_rl_repo", "/root/.axon_site/_ro/trn_rl_repo"):
    if os.path.isdir(_p) and _p not in sys.path:
        sys.path.append(_p)

import concourse.bass as bass  # noqa: E402
import concourse.bacc as bacc  # noqa: E402
import concourse.tile as tile  # noqa: E402
from concourse import mybir  # noqa: E402
from concourse.bass_utils import run_bass_kernel_spmd  # noqa: E402

B, N, F, H, L = 4096, 128, 67, 64, 4
NC = 8
BC = B // NC  # 512 batches per core
FA = 64  # features handled by the main packed tensor
FR = F - FA  # 3 remainder features

NB = 16  # batches per block
NP = NB // 2  # pairs per block
NBLK = BC // NB
F32 = mybir.dt.float32

_CACHE: dict = {}


def _build_nc(b_fc_val: float):
    nc = bacc.Bacc("TRN2", target_bir_lowering=False, debug=False, num_devices=NC)

    F16 = mybir.dt.float16
    adjT_d = nc.dram_tensor("adjT", [128, BC, 128], F32, kind="ExternalInput").ap()
    # node is shipped as a split-fp16 pair (hi + lo) so the embedding can run
    # fp16 matmuls (1 cyc/row vs fp32's 4) while staying exact to ~2^-21
    ndAh_d = nc.dram_tensor("ndAh", [128, BC // 2, 128], F16, kind="ExternalInput").ap()
    ndAl_d = nc.dram_tensor("ndAl", [128, BC // 2, 128], F16, kind="ExternalInput").ap()
    ndRh_d = nc.dram_tensor("ndRh", [2 * FR, BC // 2, 128], F16, kind="ExternalInput").ap()
    ndRl_d = nc.dram_tensor("ndRl", [2 * FR, BC // 2, 128], F16, kind="ExternalInput").ap()
    # weights, all pair-block-diagonal / broadcast-packed on the host
    webdh_d = nc.dram_tensor("webdh", [128, 128], F16, kind="ExternalInput").ap()
    webdl_d = nc.dram_tensor("webdl", [128, 128], F16, kind="ExternalInput").ap()
    wermh_d = nc.dram_tensor("wermh", [2 * FR, 128], F16, kind="ExternalInput").ap()
    werml_d = nc.dram_tensor("werml", [2 * FR, 128], F16, kind="ExternalInput").ap()
    wbd_d = nc.dram_tensor("wbd", [128, L, 128], F32, kind="ExternalInput").ap()
    bbc_d = nc.dram_tensor("bbc", [128, L, 128], F32, kind="ExternalInput").ap()
    wfc_d = nc.dram_tensor("wfc", [128, 2], F32, kind="ExternalInput").ap()
    out_d = nc.dram_tensor("out", [2, BC // 2], F32, kind="ExternalOutput").ap()

    with tile.TileContext(nc) as tc:
        with (
            tc.tile_pool(name="consts", bufs=1) as consts,
            tc.tile_pool(name="adj", bufs=4) as adj_pool,
            tc.tile_pool(name="nodes", bufs=4) as node_pool,
            tc.tile_pool(name="state", bufs=9) as state_pool,
            tc.tile_pool(name="hwb", bufs=5) as hwb_pool,
            tc.tile_pool(name="sig", bufs=5) as sig_pool,
            tc.tile_pool(name="gout", bufs=1) as gout_pool,
            tc.tile_pool(name="p_mm", bufs=4, space="PSUM") as p_mm,
            tc.tile_pool(name="p_hwb", bufs=4, space="PSUM") as p_hwb,
        ):
            webdh_t = consts.tile([128, 128], F16)
            nc.sync.dma_start(webdh_t[:], webdh_d[:])
            webdl_t = consts.tile([128, 128], F16)
            nc.sync.dma_start(webdl_t[:], webdl_d[:])
            wermh_t = consts.tile([2 * FR, 128], F16)
            nc.sync.dma_start(wermh_t[:], wermh_d[:])
            werml_t = consts.tile([2 * FR, 128], F16)
            nc.sync.dma_start(werml_t[:], werml_d[:])
            wbd_t = consts.tile([128, L, 128], F32)
            nc.sync.dma_start(wbd_t[:], wbd_d[:])
            bbc_t = consts.tile([128, L, 128], F32)
            nc.sync.dma_start(bbc_t[:], bbc_d[:])
            wfc_t = consts.tile([128, 2], F32)
            nc.sync.dma_start(wfc_t[:], wfc_d[:])
            bfc_t = consts.tile([2, 1], F32)
            nc.gpsimd.memset(bfc_t[:], float(b_fc_val))

            g_all = gout_pool.tile([128, BC // 2], F32)

            # Two DMA blocks run in lockstep, each split into 2 compute
            # chunks of NPC pairs, so the PE always has another chunk's
            # matmuls available while one chunk's elementwise epilogue
            # (DVE/ACT) runs, and chain-link latencies stay short.
            INTER = 2
            NCH = 2
            NPC = NP // NCH
            assert NBLK % INTER == 0

            def load_block(blk):
                bt = {}
                adjT = adj_pool.tile([128, NB, 128], F32, tag="adj")
                nc.sync.dma_start(adjT[:], adjT_d[:, blk * NB : (blk + 1) * NB, :])
                bt["adjT"] = adjT
                pslice = slice(blk * NP, (blk + 1) * NP)
                for name, dram, shape in (
                    ("ndAh", ndAh_d, [128, NP, 128]),
                    ("ndAl", ndAl_d, [128, NP, 128]),
                    ("ndRh", ndRh_d, [2 * FR, NP, 128]),
                    ("ndRl", ndRl_d, [2 * FR, NP, 128]),
                ):
                    t = node_pool.tile(shape, F16, tag=name)
                    nc.sync.dma_start(t[:], dram[:, pslice, :])
                    bt[name] = t
                return bt

            def emb_mm(st):
                bt, qs = st["bt"], st["qs"]
                embp = p_hwb.tile([128, NPC, 128], F32, tag="hwbp")
                # 3-term split-fp16 product per operand pair (lo*lo dropped)
                out = embp[:, :, :]
                nc.tensor.matmul(out, webdh_t[:], bt["ndAh"][:, qs, :], start=True, stop=False)
                nc.tensor.matmul(out, webdh_t[:], bt["ndAl"][:, qs, :], start=False, stop=False)
                nc.tensor.matmul(out, webdl_t[:], bt["ndAh"][:, qs, :], start=False, stop=False)
                nc.tensor.matmul(out, wermh_t[:], bt["ndRh"][:, qs, :], start=False, stop=False)
                nc.tensor.matmul(out, wermh_t[:], bt["ndRl"][:, qs, :], start=False, stop=False)
                nc.tensor.matmul(out, werml_t[:], bt["ndRh"][:, qs, :], start=False, stop=True)
                st["embp"] = embp

            def emb_copy(st):
                s = state_pool.tile([128, NPC, 128], F32, tag="state")
                nc.scalar.copy(s[:], st["embp"][:])
                st["s"] = s
                del st["embp"]

            def mm1(st, l):
                hwbp = p_hwb.tile([128, NPC, 128], F32, tag="hwbp")
                for p in range(NPC):
                    nc.tensor.matmul(hwbp[:, p, :], st["s"][:, p, :], wbd_t[:, l, :])
                st["hwbp"] = hwbp

            def copy1(st, l):
                hwbs = hwb_pool.tile([128, NPC, 128], F32, tag="hwbs")
                bbc_b = bbc_t[:, l, :].rearrange(
                    "p (one n) -> p one n", one=1
                ).broadcast_to((128, NPC, 128))
                nc.vector.tensor_tensor(
                    hwbs[:], st["hwbp"][:], bbc_b, op=mybir.AluOpType.add
                )
                st["hwbs"] = hwbs
                del st["hwbp"]

            def mm2(st, l):
                msgp = p_mm.tile([128, NPC, 128], F32, tag="mm")
                hwbs, adjT, po = st["hwbs"], st["bt"]["adjT"], st["po"]
                for p in range(NPC):
                    nc.tensor.matmul(
                        msgp[0:64, p, :], hwbs[:, p, 0:64], adjT[:, 2 * (po + p), :]
                    )
                    nc.tensor.matmul(
                        msgp[64:128, p, :], hwbs[:, p, 64:128],
                        adjT[:, 2 * (po + p) + 1, :],
                    )
                st["msgp"] = msgp

            def epilogue(st, l):
                sig = sig_pool.tile([128, NPC, 128], F32, tag="sig")
                nc.scalar.activation(
                    sig[:], st["msgp"][:], mybir.ActivationFunctionType.Sigmoid
                )
                del st["msgp"]
                s2 = state_pool.tile([128, NPC, 128], F32, tag="state")
                nc.vector.tensor_add(s2[:], sig[:], st["s"][:])
                if l == 0:
                    nc.vector.tensor_scalar_max(s2[:], s2[:], 0.0)
                st["s"] = s2

            def readout(st):
                g0 = st["gp"] * NPC
                nc.vector.tensor_reduce(
                    g_all[:, g0 : g0 + NPC], st["s"][:],
                    axis=mybir.AxisListType.X, op=mybir.AluOpType.add,
                )

            for bp in range(NBLK // INTER):
                blks = [bp * INTER + j for j in range(INTER)]
                bts = [load_block(blk) for blk in blks]
                sts = []
                for blk, bt in zip(blks, bts):
                    for ch in range(NCH):
                        sts.append(
                            {
                                "bt": bt,
                                "qs": slice(ch * NPC, (ch + 1) * NPC),
                                "po": ch * NPC,
                                "gp": blk * NCH + ch,
                            }
                        )
                for st in sts:
                    emb_mm(st)
                for st in sts:
                    emb_copy(st)
                for l in range(L):
                    for st in sts:
                        mm1(st, l)
                        copy1(st, l)
                    for st in sts:
                        mm2(st, l)
                        epilogue(st, l)
                for st in sts:
                    readout(st)

            # ---- final FC + sigmoid for all batches of this core
            fcp = p_mm.tile([2, BC // 2], F32, tag="mm")
            nc.tensor.matmul(fcp[:], wfc_t[:], g_all[:])
            out_t = gout_pool.tile([2, BC // 2], F32)
            nc.scalar.activation(
                out_t[:], fcp[:], mybir.ActivationFunctionType.Sigmoid,
                bias=bfc_t[:],
            )
            nc.sync.dma_start(out_d[:], out_t[:])

    nc.compile()
    return nc


def _split16(x):
    hi = x.astype(np.float16)
    lo = (x - hi.astype(np.float32)).astype(np.float16)
    return hi, lo


def _pack_weights(W_emb, W_gcn, b_gcn, W_fc):
    webd = np.zeros((128, 128), np.float32)
    webd[0:FA, 0:H] = W_emb[0:FA]
    webd[FA : 2 * FA, H : 2 * H] = W_emb[0:FA]
    werm = np.zeros((2 * FR, 128), np.float32)
    werm[0:FR, 0:H] = W_emb[FA:F]
    werm[FR : 2 * FR, H : 2 * H] = W_emb[FA:F]
    webdh, webdl = _split16(webd)
    wermh, werml = _split16(werm)
    wbd = np.zeros((128, L, 128), np.float32)
    for l in range(L):
        wbd[0:H, l, 0:H] = W_gcn[l]
        wbd[H : 2 * H, l, H : 2 * H] = W_gcn[l]
    brow = np.concatenate([b_gcn, b_gcn], axis=1)  # [L, 128]
    bbc = np.broadcast_to(brow[None, :, :], (128, L, 128)).copy()
    wfc = np.zeros((128, 2), np.float32)
    wfc[0:H, 0] = W_fc[:, 0]
    wfc[H : 2 * H, 1] = W_fc[:, 0]
    return webdh, webdl, wermh, werml, wbd, bbc, wfc


def _make_in_maps(node, adj, W_emb, W_gcn, b_gcn, W_fc):
    webdh, webdl, wermh, werml, wbd, bbc, wfc = _pack_weights(W_emb, W_gcn, b_gcn, W_fc)
    in_maps = []
    for c in range(NC):
        node_c = node[c * BC : (c + 1) * BC]  # [BC, N, F]
        adj_c = adj[c * BC : (c + 1) * BC]  # [BC, N, N]
        # adjT[m, b, n] = adj_c[b, n, m]
        adjT = np.ascontiguousarray(adj_c.transpose(2, 0, 1))
        # nodeT[f, b, n] = node_c[b, n, f]
        nodeT = node_c.transpose(2, 0, 1)
        ndA = np.ascontiguousarray(
            np.concatenate([nodeT[0:FA, 0::2, :], nodeT[0:FA, 1::2, :]], axis=0)
        )
        ndR = np.ascontiguousarray(
            np.concatenate([nodeT[FA:F, 0::2, :], nodeT[FA:F, 1::2, :]], axis=0)
        )
        ndAh, ndAl = _split16(ndA)
        ndRh, ndRl = _split16(ndR)
        in_maps.append(
            {
                "adjT": adjT,
                "ndAh": ndAh,
                "ndAl": ndAl,
                "ndRh": ndRh,
                "ndRl": ndRl,
                "webdh": webdh,
                "webdl": webdl,
                "wermh": wermh,
                "werml": werml,
                "wbd": wbd,
                "bbc": bbc,
                "wfc": wfc,
            }
        )
    return in_maps


def kernel(node, adj, W_emb, W_gcn, b_gcn, W_fc, b_fc):
    node = np.ascontiguousarray(np.asarray(node, np.float32))
    adj = np.ascontiguousarray(np.asarray(adj, np.float32))
    W_emb = np.asarray(W_emb, np.float32)
    W_gcn = np.asarray(W_gcn, np.float32)
    b_gcn = np.asarray(b_gcn, np.float32)
    W_fc = np.asarray(W_fc, np.float32)
    b_fc = np.asarray(b_fc, np.float32)

    key = ("nc", float(b_fc[0]))
    if key not in _CACHE:
        _CACHE[key] = _build_nc(float(b_fc[0]))
    nc = _CACHE[key]

    in_maps = _make_in_maps(node, adj, W_emb, W_gcn, b_gcn, W_fc)
    _CACHE["in_maps"] = in_maps
    res = run_bass_kernel_spmd(nc, in_maps, list(range(NC))).results

    out = np.empty((B,), np.float32)
    for c in range(NC):
        o = res[c]["out"]  # [2, BC//2]
        out[c * BC : (c + 1) * BC : 2] = o[0]
        out[c * BC + 1 : (c + 1) * BC : 2] = o[1]
    return out


# revision 29
# speedup vs baseline: 1.0198x; 1.0198x over previous
"""Trainium2 Bass kernel for a 4-layer GCN (nn_GCNModel_44762149159246).

Reference math (per batch b of B=4096, N=128 nodes, F=67 in-feats, H=64):
    h0 = node @ W_emb                                  [N, H]
    for l in 0..3:  h = relu(sigmoid(adj @ (h W_l + b_l)) + h)
    out = sigmoid(sum_n h @ W_fc + b_fc)               scalar per batch

Sharding: pure data-parallel over the batch dim, 512 batches per core x 8.

Device-side layout (everything transpose-free on device):
  - state is kept TRANSPOSED and PAIR-PACKED: tile [128, n] where partitions
    0..63 hold features of the even batch of a pair and 64..127 the odd batch.
  - adj is passed host-transposed per batch ([m, b, n] so that adj.T tiles
    stream naturally as the matmul moving operand).
  - node is passed pair-packed/transposed ([feat-pair, pair, n]) with the
    3 leftover features (67 = 64 + 3) in a small side tensor, contracted via
    a second accumulating matmul.
  - per-layer W is a 128x128 block-diagonal [[W, 0], [0, W]] so one matmul
    computes both halves of a pair; the bias is added during the PSUM->SBUF
    copy with a broadcast tile.
  - relu is provably a no-op for layers 1..3 (sigmoid > 0 and h >= 0), so it
    is only applied after layer 0.
"""

import os
import sys

import numpy as np

for _p in (
    "/opt/trn_rl_repo",
    "/root/.axon_site/_ro/trn_rl_repo",
    "/root/.axon_site/_ro/pypackages",
):
    if os.path.isdir(_p) and _p not in sys.path:
        sys.path.append(_p)

import concourse.bass as bass  # noqa: E402
import concourse.bacc as bacc  # noqa: E402
import concourse.tile as tile  # noqa: E402
from concourse import mybir  # noqa: E402
from concourse.bass_utils import run_bass_kernel_spmd  # noqa: E402

B, N, F, H, L = 4096, 128, 67, 64, 4
NC = 8
BC = B // NC  # 512 batches per core
FA = 64  # features handled by the main packed tensor
FR = F - FA  # 3 remainder features

NB = 16  # batches per block
NP = NB // 2  # pairs per block
NBLK = BC // NB
F32 = mybir.dt.float32

_CACHE: dict = {}


def _build_nc(b_fc_val: float):
    nc = bacc.Bacc("TRN2", target_bir_lowering=False, debug=False, num_devices=NC)

    F16 = mybir.dt.float16
    adjT_d = nc.dram_tensor("adjT", [128, BC, 128], F32, kind="ExternalInput").ap()
    # node is shipped as a split-fp16 pair (hi + lo) so the embedding can run
    # fp16 matmuls (1 cyc/row vs fp32's 4) while staying exact to ~2^-21
    ndAh_d = nc.dram_tensor("ndAh", [128, BC // 2, 128], F16, kind="ExternalInput").ap()
    ndAl_d = nc.dram_tensor("ndAl", [128, BC // 2, 128], F16, kind="ExternalInput").ap()
    ndRh_d = nc.dram_tensor("ndRh", [2 * FR, BC // 2, 128], F16, kind="ExternalInput").ap()
    ndRl_d = nc.dram_tensor("ndRl", [2 * FR, BC // 2, 128], F16, kind="ExternalInput").ap()
    # weights, all pair-block-diagonal / broadcast-packed on the host
    webdh_d = nc.dram_tensor("webdh", [128, 128], F16, kind="ExternalInput").ap()
    webdl_d = nc.dram_tensor("webdl", [128, 128], F16, kind="ExternalInput").ap()
    wermh_d = nc.dram_tensor("wermh", [2 * FR, 128], F16, kind="ExternalInput").ap()
    werml_d = nc.dram_tensor("werml", [2 * FR, 128], F16, kind="ExternalInput").ap()
    wbd_d = nc.dram_tensor("wbd", [128, L, 128], F32, kind="ExternalInput").ap()
    bbc_d = nc.dram_tensor("bbc", [128, L, 128], F32, kind="ExternalInput").ap()
    wfc_d = nc.dram_tensor("wfc", [128, 2], F32, kind="ExternalInput").ap()
    out_d = nc.dram_tensor("out", [2, BC // 2], F32, kind="ExternalOutput").ap()

    with tile.TileContext(nc) as tc:
        with (
            tc.tile_pool(name="consts", bufs=1) as consts,
            tc.tile_pool(name="adj", bufs=6) as adj_pool,
            tc.tile_pool(name="nodes", bufs=6) as node_pool,
            tc.tile_pool(name="state", bufs=17) as state_pool,
            tc.tile_pool(name="hwb", bufs=9) as hwb_pool,
            tc.tile_pool(name="sig", bufs=9) as sig_pool,
            tc.tile_pool(name="gout", bufs=1) as gout_pool,
            tc.tile_pool(name="p_mm", bufs=4, space="PSUM") as p_mm,
            tc.tile_pool(name="p_hwb", bufs=4, space="PSUM") as p_hwb,
        ):
            webdh_t = consts.tile([128, 128], F16)
            nc.sync.dma_start(webdh_t[:], webdh_d[:])
            webdl_t = consts.tile([128, 128], F16)
            nc.sync.dma_start(webdl_t[:], webdl_d[:])
            wermh_t = consts.tile([2 * FR, 128], F16)
            nc.sync.dma_start(wermh_t[:], wermh_d[:])
            werml_t = consts.tile([2 * FR, 128], F16)
            nc.sync.dma_start(werml_t[:], werml_d[:])
            wbd_t = consts.tile([128, L, 128], F32)
            nc.sync.dma_start(wbd_t[:], wbd_d[:])
            bbc_t = consts.tile([128, L, 128], F32)
            nc.sync.dma_start(bbc_t[:], bbc_d[:])
            wfc_t = consts.tile([128, 2], F32)
            nc.sync.dma_start(wfc_t[:], wfc_d[:])
            bfc_t = consts.tile([2, 1], F32)
            nc.gpsimd.memset(bfc_t[:], float(b_fc_val))

            g_all = gout_pool.tile([128, BC // 2], F32)

            # Two DMA blocks run in lockstep, each split into 2 compute
            # chunks of NPC pairs, so the PE always has another chunk's
            # matmuls available while one chunk's elementwise epilogue
            # (DVE/ACT) runs, and chain-link latencies stay short.
            INTER = 4
            NCH = 2
            NPC = NP // NCH
            assert NBLK % INTER == 0

            def load_block(blk):
                # node tensors first: the embedding only needs these, so the
                # PE can start while the bigger adjacency DMA streams in
                bt = {}
                pslice = slice(blk * NP, (blk + 1) * NP)
                for name, dram, shape in (
                    ("ndAh", ndAh_d, [128, NP, 128]),
                    ("ndAl", ndAl_d, [128, NP, 128]),
                    ("ndRh", ndRh_d, [2 * FR, NP, 128]),
                    ("ndRl", ndRl_d, [2 * FR, NP, 128]),
                ):
                    t = node_pool.tile(shape, F16, tag=name)
                    nc.sync.dma_start(t[:], dram[:, pslice, :])
                    bt[name] = t
                adjT = adj_pool.tile([128, NB, 128], F32, tag="adj")
                nc.sync.dma_start(adjT[:], adjT_d[:, blk * NB : (blk + 1) * NB, :])
                bt["adjT"] = adjT
                return bt

            def emb_mm(st):
                bt, qs = st["bt"], st["qs"]
                embp = p_hwb.tile([128, NPC, 128], F32, tag="hwbp")
                # 3-term split-fp16 product per operand pair (lo*lo dropped)
                out = embp[:, :, :]
                nc.tensor.matmul(out, webdh_t[:], bt["ndAh"][:, qs, :], start=True, stop=False)
                nc.tensor.matmul(out, webdh_t[:], bt["ndAl"][:, qs, :], start=False, stop=False)
                nc.tensor.matmul(out, webdl_t[:], bt["ndAh"][:, qs, :], start=False, stop=False)
                nc.tensor.matmul(out, wermh_t[:], bt["ndRh"][:, qs, :], start=False, stop=False)
                nc.tensor.matmul(out, wermh_t[:], bt["ndRl"][:, qs, :], start=False, stop=False)
                nc.tensor.matmul(out, werml_t[:], bt["ndRh"][:, qs, :], start=False, stop=True)
                st["embp"] = embp

            def emb_copy(st):
                s = state_pool.tile([128, NPC, 128], F32, tag="state")
                nc.scalar.copy(s[:], st["embp"][:])
                st["s"] = s
                del st["embp"]

            def mm1(st, l):
                hwbp = p_hwb.tile([128, NPC, 128], F32, tag="hwbp")
                for p in range(NPC):
                    nc.tensor.matmul(hwbp[:, p, :], st["s"][:, p, :], wbd_t[:, l, :])
                st["hwbp"] = hwbp

            def copy1(st, l):
                hwbs = hwb_pool.tile([128, NPC, 128], F32, tag="hwbs")
                bbc_b = bbc_t[:, l, :].rearrange(
                    "p (one n) -> p one n", one=1
                ).broadcast_to((128, NPC, 128))
                nc.vector.tensor_tensor(
                    hwbs[:], st["hwbp"][:], bbc_b, op=mybir.AluOpType.add
                )
                st["hwbs"] = hwbs
                del st["hwbp"]

            def mm2(st, l):
                msgp = p_mm.tile([128, NPC, 128], F32, tag="mm")
                hwbs, adjT, po = st["hwbs"], st["bt"]["adjT"], st["po"]
                for p in range(NPC):
                    nc.tensor.matmul(
                        msgp[0:64, p, :], hwbs[:, p, 0:64], adjT[:, 2 * (po + p), :]
                    )
                    nc.tensor.matmul(
                        msgp[64:128, p, :], hwbs[:, p, 64:128],
                        adjT[:, 2 * (po + p) + 1, :],
                    )
                st["msgp"] = msgp

            def epilogue(st, l):
                sig = sig_pool.tile([128, NPC, 128], F32, tag="sig")
                nc.scalar.activation(
                    sig[:], st["msgp"][:], mybir.ActivationFunctionType.Sigmoid
                )
                del st["msgp"]
                s2 = state_pool.tile([128, NPC, 128], F32, tag="state")
                nc.vector.tensor_add(s2[:], sig[:], st["s"][:])
                if l == 0:
                    nc.vector.tensor_scalar_max(s2[:], s2[:], 0.0)
                st["s"] = s2

            def readout(st):
                g0 = st["gp"] * NPC
                nc.vector.tensor_reduce(
                    g_all[:, g0 : g0 + NPC], st["s"][:],
                    axis=mybir.AxisListType.X, op=mybir.AluOpType.add,
                )

            for bp in range(NBLK // INTER):
                blks = [bp * INTER + j for j in range(INTER)]
                bts = [load_block(blk) for blk in blks]
                sts = []
                for blk, bt in zip(blks, bts):
                    for ch in range(NCH):
                        sts.append(
                            {
                                "bt": bt,
                                "qs": slice(ch * NPC, (ch + 1) * NPC),
                                "po": ch * NPC,
                                "gp": blk * NCH + ch,
                            }
                        )
                for st in sts:
                    emb_mm(st)
                for st in sts:
                    emb_copy(st)
                for l in range(L):
                    for st in sts:
                        mm1(st, l)
                        copy1(st, l)
                    for st in sts:
                        mm2(st, l)
                        epilogue(st, l)
                for st in sts:
                    readout(st)

            # ---- final FC + sigmoid for all batches of this core
            fcp = p_mm.tile([2, BC // 2], F32, tag="mm")
            nc.tensor.matmul(fcp[:], wfc_t[:], g_all[:])
            out_t = gout_pool.tile([2, BC // 2], F32)
            nc.scalar.activation(
                out_t[:], fcp[:], mybir.ActivationFunctionType.Sigmoid,
                bias=bfc_t[:],
            )
            nc.sync.dma_start(out_d[:], out_t[:])

    nc.compile()
    return nc


def _split16(x):
    hi = x.astype(np.float16)
    lo = (x - hi.astype(np.float32)).astype(np.float16)
    return hi, lo


def _pack_weights(W_emb, W_gcn, b_gcn, W_fc):
    webd = np.zeros((128, 128), np.float32)
    webd[0:FA, 0:H] = W_emb[0:FA]
    webd[FA : 2 * FA, H : 2 * H] = W_emb[0:FA]
    werm = np.zeros((2 * FR, 128), np.float32)
    werm[0:FR, 0:H] = W_emb[FA:F]
    werm[FR : 2 * FR, H : 2 * H] = W_emb[FA:F]
    webdh, webdl = _split16(webd)
    wermh, werml = _split16(werm)
    wbd = np.zeros((128, L, 128), np.float32)
    for l in range(L):
        wbd[0:H, l, 0:H] = W_gcn[l]
        wbd[H : 2 * H, l, H : 2 * H] = W_gcn[l]
    brow = np.concatenate([b_gcn, b_gcn], axis=1)  # [L, 128]
    bbc = np.broadcast_to(brow[None, :, :], (128, L, 128)).copy()
    wfc = np.zeros((128, 2), np.float32)
    wfc[0:H, 0] = W_fc[:, 0]
    wfc[H : 2 * H, 1] = W_fc[:, 0]
    return webdh, webdl, wermh, werml, wbd, bbc, wfc


def _make_in_maps(node, adj, W_emb, W_gcn, b_gcn, W_fc):
    webdh, webdl, wermh, werml, wbd, bbc, wfc = _pack_weights(W_emb, W_gcn, b_gcn, W_fc)
    in_maps = []
    for c in range(NC):
        node_c = node[c * BC : (c + 1) * BC]  # [BC, N, F]
        adj_c = adj[c * BC : (c + 1) * BC]  # [BC, N, N]
        # adjT[m, b, n] = adj_c[b, n, m]
        adjT = np.ascontiguousarray(adj_c.transpose(2, 0, 1))
        # nodeT[f, b, n] = node_c[b, n, f]
        nodeT = node_c.transpose(2, 0, 1)
        ndA = np.ascontiguousarray(
            np.concatenate([nodeT[0:FA, 0::2, :], nodeT[0:FA, 1::2, :]], axis=0)
        )
        ndR = np.ascontiguousarray(
            np.concatenate([nodeT[FA:F, 0::2, :], nodeT[FA:F, 1::2, :]], axis=0)
        )
        ndAh, ndAl = _split16(ndA)
        ndRh, ndRl = _split16(ndR)
        in_maps.append(
            {
                "adjT": adjT,
                "ndAh": ndAh,
                "ndAl": ndAl,
                "ndRh": ndRh,
                "ndRl": ndRl,
                "webdh": webdh,
                "webdl": webdl,
                "wermh": wermh,
                "werml": werml,
                "wbd": wbd,
                "bbc": bbc,
                "wfc": wfc,
            }
        )
    return in_maps


def kernel(node, adj, W_emb, W_gcn, b_gcn, W_fc, b_fc):
    node = np.ascontiguousarray(np.asarray(node, np.float32))
    adj = np.ascontiguousarray(np.asarray(adj, np.float32))
    W_emb = np.asarray(W_emb, np.float32)
    W_gcn = np.asarray(W_gcn, np.float32)
    b_gcn = np.asarray(b_gcn, np.float32)
    W_fc = np.asarray(W_fc, np.float32)
    b_fc = np.asarray(b_fc, np.float32)

    key = ("nc", float(b_fc[0]))
    if key not in _CACHE:
        _CACHE[key] = _build_nc(float(b_fc[0]))
    nc = _CACHE[key]

    in_maps = _make_in_maps(node, adj, W_emb, W_gcn, b_gcn, W_fc)
    _CACHE["in_maps"] = in_maps
    res = run_bass_kernel_spmd(nc, in_maps, list(range(NC))).results

    out = np.empty((B,), np.float32)
    for c in range(NC):
        o = res[c]["out"]  # [2, BC//2]
        out[c * BC : (c + 1) * BC : 2] = o[0]
        out[c * BC + 1 : (c + 1) * BC : 2] = o[1]
    return out
